# revision 1
# baseline (speedup 1.0000x reference)
"""KalmanNetNN single-step kernel for 8x TRN2 NeuronCores (Bass/Tile).

Data-parallel: batch 65536 split across 8 cores (8192 rows each), processed
in 16 tiles of 512 batch rows (batch on the free dim, features on partitions).

Layout (SBUF partition ranges start at a legal zone base 0/32/64/96; both
SBUF inputs of a 2-tensor vector op share the same base; outputs may remap):
  A1T [128,512] = transpose-DMA of [h_Q | h_Sigma]
  A2T [128,512] = [h_S 0:64 | y_prev 64:72 | xp_prev 96:104 | xpriorp 104:112]
  SMT [128,512] = [xp_hi 0:8, xp_lo 8:16 | y 64:72 | xp 96:104, xp 104:112]
  D   [128,512] bf16 diffs (memset 0): {oid 0:8, fed 32:40, fud 40:48, od 64:72}

Accumulating matmul groups must share one tile_position (different row-group
positions run concurrently on distinct 32x32 sub-arrays and collide on the
same PSUM accumulator -> device fault).  Pieces whose rhs lives at base 64
therefore use full-height [128, M] lhsT blocks (zero rows outside the real
input rows) so every matmul in a group sits at tp0=0.  K-padding is free:
matmul time is N cycles regardless of K.

L2-normalize uses one fused sumsq matmul per tile plus a single batched
Ln + Exp(-0.5 x) on ACT across all tiles (one activation-table set; the
direct Rsqrt ACT function is banned for accuracy).
GRU: r,z from one fused [128,512] psum (r 0:64, z 64:128);
n = tanh(Wih_n x + bih_n + r*(Whh_n h + bhh_n)) via one STT + one TT.
fc3/fc4 are skipped: the graded output (new m1x_posterior) ignores them.
"""

import sys
import numpy as np
import ml_dtypes

sys.path.insert(0, "/opt/trn_rl_repo")

B_FULL = 65536
NCORES = 8
BC = B_FULL // NCORES      # rows per core
BF = 512                   # batch tile (free dim)
BF16_NP = ml_dtypes.bfloat16

_cached = {}


def _bf16(x):
    return np.asarray(x, dtype=np.float32).astype(BF16_NP)


class _WImg:
    """Host-side SBUF weight image: [128, ncols], 16-element col alignment."""

    def __init__(self, np_dtype):
        self.np_dtype = np_dtype
        self.cols = 0
        self.blocks = {}   # name -> (row0, nrows, col0, ncols)
        self.data = []

    def place(self, name, row0, arr):
        arr = np.asarray(arr, dtype=self.np_dtype)
        k, m = arr.shape
        col0 = (self.cols + 15) // 16 * 16
        self.cols = col0 + m
        self.blocks[name] = (row0, k, col0, m)
        self.data.append((row0, col0, arr))
        return name

    def image(self):
        ncols = (self.cols + 15) // 16 * 16
        img = np.zeros((128, ncols), dtype=self.np_dtype)
        for row0, col0, arr in self.data:
            k, m = arr.shape
            img[row0:row0 + k, col0:col0 + m] = arr
        return img


def _prep_weights(inp):
    f64 = np.float64
    F = np.asarray(inp["F_mat"], f64)
    H = np.asarray(inp["H_mat"], f64)
    HF = H @ F

    def hi_lo(a):
        hi = _bf16(a).astype(np.float32)
        lo = _bf16(np.asarray(a, np.float32) - hi)
        return _bf16(hi), lo

    F_hi, F_lo = hi_lo(F)
    HF_hi, HF_lo = hi_lo(HF)

    wb = _WImg(BF16_NP)
    wb.place("m1y_a", 0, np.concatenate([np.asarray(HF_hi, f64).T,
                                         np.asarray(HF_hi, f64).T], axis=0))
    wb.place("m1y_b", 0, np.asarray(HF_lo, f64).T)
    wb.place("prior_a", 0, np.concatenate([np.asarray(F_hi, f64).T,
                                           np.asarray(F_hi, f64).T], axis=0))
    wb.place("prior_b", 0, np.asarray(F_lo, f64).T)

    # fused sumsq: one [128,65] lhsT; ss rows {0:32 oid, 32 fed, 33 fud, 34:65 od}
    m2 = np.zeros((128, 65))
    m2[0:8, 0:32] = 1.0
    m2[32:40, 32] = 1.0
    m2[40:48, 33] = 1.0
    m2[64:72, 34:65] = 1.0
    wb.place("mm2", 0, m2)
    # fused scale replication: [65,128] lhsT -> rs rows
    #  {0:32 <- s_oid, 32:40 <- s_fed, 40:64 <- s_fud, 64:128 <- s_od}
    m3 = np.zeros((65, 128))
    m3[0, 0:32] = 1.0
    m3[32, 32:40] = 1.0
    m3[33, 40:64] = 1.0
    m3[34, 64:128] = 1.0
    wb.place("mm3", 0, m3)
    # FC5+FC6 fused: rhs nd[0:128]; out {0:64 FC5(pad), 64:128 FC6(pad)}
    fc5 = np.asarray(inp["fc5_w"], f64)
    fc6 = np.asarray(inp["fc6_w"], f64)
    fc7 = np.asarray(inp["fc7_w"], f64)   # cols 0:8 od, 8:16 oid
    w56 = np.zeros((128, 128))
    w56[32:40, 0:40] = fc5.T
    w56[40:48, 64:104] = fc6.T
    wb.place("f56", 0, w56)
    w7 = np.zeros((128, 128))
    w7[64:72, 0:80] = fc7[:, 0:8].T       # od-hat part
    w7[0:8, 0:80] = fc7[:, 8:16].T        # oid-hat part
    wb.place("f7", 0, w7)

    # GRU weights; gate order r,z (rows 0:128) and n (rows 128:192).
    # Pieces whose rhs sits above base 0 get full-height zero-padded blocks.
    def padded(rows0, w):     # w: [K, M] real rows placed at rows0
        out = np.zeros((128, w.shape[1]))
        out[rows0:rows0 + w.shape[0]] = w
        return out

    for g, xspec in (
        ("Q", [("x", "gruQ_Wih", 0, 40, 0)]),
        ("Sig", [("x1", "gruSig_Wih", 0, 64, 0),
                 ("x2", "gruSig_Wih", 64, 104, 64)]),
        ("S", [("x1", "gruS_Wih", 0, 64, 0), ("x2", "gruS_Wih", 64, 144, 0)]),
    ):
        whh = np.asarray(inp[f"gru{g}_Whh"], f64)
        if g == "Sig":   # h_Sigma lives at a1[64:128] -> full-height block
            wb.place(f"{g}_rz_h", 0, padded(64, whh[0:128].T))
            wb.place(f"{g}_n_h", 0, padded(64, whh[128:192].T))
        else:
            wb.place(f"{g}_rz_h", 0, whh[0:128].T)
            wb.place(f"{g}_n_h", 0, whh[128:192].T)
        for tag, wname, c0, c1, rbase in xspec:
            wih = np.asarray(inp[wname], f64)
            piece = wih[:, c0:c1]
            if rbase:        # out_FC6 lives at x56[64:104] -> full-height
                wb.place(f"{g}_rz_{tag}", 0, padded(rbase, piece[0:128].T))
                wb.place(f"{g}_n_{tag}", 0, padded(rbase, piece[128:192].T))
            else:
                wb.place(f"{g}_rz_{tag}", 0, piece[0:128].T)
                wb.place(f"{g}_n_{tag}", 0, piece[128:192].T)

    wb.place("fc1", 0, np.asarray(inp["fc1_w"], f64).T)
    w1 = np.asarray(inp["fc2_w1"], f64)
    w2 = np.asarray(inp["fc2_w2"], f64)
    for c in range(4):
        wb.place(f"fc2a_sig{c}", 0, w1[128 * c:128 * (c + 1), 0:64].T)
        wb.place(f"fc2a_s{c}", 0, w1[128 * c:128 * (c + 1), 64:128].T)
        wb.place(f"fc2b{c}", 0, w2[:, 128 * c:128 * (c + 1)].T)
    dyr = np.zeros((8, 64))
    for m in range(64):
        dyr[m % 8, m] = 1.0
    wb.place("dyrep", 0, dyr)
    fin = np.zeros((64, 8))
    for m in range(64):
        fin[m, m // 8] = 1.0
    wb.place("final", 0, fin)

    wf = _WImg(np.float32)
    for g in ("Q", "Sig", "S"):
        bih = np.asarray(inp[f"gru{g}_bih"], f64)
        bhh = np.asarray(inp[f"gru{g}_bhh"], f64)
        wf.place(f"rzb_{g}", 0, (bih[0:128] + bhh[0:128])[:, None])
        wf.place(f"nb_{g}", 0, bih[128:192][:, None])
        wf.place(f"bhhn_{g}", 0, bhh[128:192][:, None])
    f56b = np.zeros((128, 1))
    f56b[0:40, 0] = np.asarray(inp["fc5_b"], f64)
    f56b[64:104, 0] = np.asarray(inp["fc6_b"], f64)
    wf.place("f56b", 0, f56b)
    f7b = np.zeros((128, 1))
    f7b[0:80, 0] = np.asarray(inp["fc7_b"], f64)
    wf.place("f7b", 0, f7b)
    wf.place("f1b", 0, np.asarray(inp["fc1_b"], f64)[:, None])
    b1 = np.asarray(inp["fc2_b1"], f64)
    for c in range(4):
        wf.place(f"hidb{c}", 0, b1[128 * c:128 * (c + 1)][:, None])
    wf.place("b2", 0, np.asarray(inp["fc2_b2"], f64)[:, None])
    wf.place("ident8", 0, np.eye(8))
    return wb, wf


def _prep_batch(inp, lo, hi):
    def g(name):
        return np.asarray(inp[name][lo:hi], np.float32)

    n = hi - lo
    hq = g("h_Q"); hsig = g("h_Sigma"); hs = g("h_S")
    y = g("y")[:, :, 0]; yp = g("y_previous")[:, :, 0]
    xp = g("m1x_posterior")[:, :, 0]
    xpp = g("m1x_posterior_previous")[:, :, 0]
    xprp = g("m1x_prior_previous")[:, :, 0]
    xp_hi32 = _bf16(xp).astype(np.float32)
    xp_lo = _bf16(xp - xp_hi32)
    xp_hi = xp_hi32.astype(BF16_NP)

    a1 = np.concatenate([hq, hsig], axis=1).astype(BF16_NP)
    a2 = np.zeros((n, 128), dtype=BF16_NP)
    a2[:, 0:64] = _bf16(hs)
    a2[:, 64:72] = _bf16(yp)
    a2[:, 96:104] = _bf16(xpp)
    a2[:, 104:112] = _bf16(xprp)
    sm = np.zeros((n, 128), dtype=BF16_NP)
    sm[:, 0:8] = xp_hi
    sm[:, 8:16] = xp_lo
    sm[:, 64:72] = _bf16(y)
    sm[:, 96:104] = xp_hi
    sm[:, 104:112] = xp_hi
    return a1, a2, sm


def build(bc, wb, wf, stage=99, repeat=1):
    import concourse.bacc as bacc
    import concourse.mybir as mybir
    import concourse.tile as tile

    BF16 = mybir.dt.bfloat16
    F32 = mybir.dt.float32
    AF = mybir.ActivationFunctionType
    AL = mybir.AluOpType

    nt = bc // BF
    wbi = wb.image()
    wfi = wf.image()

    nc = bacc.Bacc()
    A1 = nc.dram_tensor("A1", [bc, 128], BF16, kind="ExternalInput")
    A2 = nc.dram_tensor("A2", [bc, 128], BF16, kind="ExternalInput")
    SM = nc.dram_tensor("SM", [bc, 128], BF16, kind="ExternalInput")
    WB = nc.dram_tensor("WB", [128, wbi.shape[1]], BF16, kind="ExternalInput")
    WF = nc.dram_tensor("WF", [128, wfi.shape[1]], F32, kind="ExternalInput")
    OUT = nc.dram_tensor("OUT", [bc, 8, 1], F32, kind="ExternalOutput")

    with tile.TileContext(nc) as tc:
        with (
            tc.tile_pool(name="wpool", bufs=1) as wpool,
            tc.tile_pool(name="inA", bufs=nt) as inA,
            tc.tile_pool(name="sbT", bufs=3) as sbT,
            tc.tile_pool(name="sbH", bufs=6) as sbH,
            tc.tile_pool(name="sbO", bufs=3) as sbO,
            tc.tile_pool(name="norm", bufs=1) as nrm,
            tc.tile_pool(name="ps", bufs=7, space="PSUM") as ps,
            tc.tile_pool(name="psT", bufs=1, space="PSUM") as psT,
        ):
            wbt = wpool.tile([128, wbi.shape[1]], BF16, tag="wbt")
            wft = wpool.tile([128, wfi.shape[1]], F32, tag="wft")
            nc.sync.dma_start(out=wbt[:], in_=WB[:])
            nc.sync.dma_start(out=wft[:], in_=WF[:])

            def W(name):
                r0, k, c0, m = wb.blocks[name]
                return wbt[r0:r0 + k, c0:c0 + m]

            def Bv(name):
                r0, k, c0, m = wf.blocks[name]
                return wft[r0:r0 + k, c0:c0 + 1]

            idc = wf.blocks["ident8"][2]

            def tail(b0, fps):
                """fps psum rows 0:8 -> fp32 transpose -> OUT[b0:b0+BF]."""
                ot = sbO.tile([8, BF], F32, tag="ot")
                nc.scalar.activation(ot[0:8, :], fps[0:8, :], AF.Copy)
                ott = psT.tile([128, 32], F32, tag="ott")
                for c in range(4):
                    nc.tensor.transpose(ott[0:128, 8 * c:8 * c + 8],
                                        ot[0:8, 128 * c:128 * (c + 1)],
                                        wft[0:8, idc:idc + 8])
                ob = sbO.tile([128, 32], F32, tag="ob")
                nc.vector.tensor_copy(ob[0:128, :], ott[0:128, :])
                dst = OUT[b0:b0 + BF, :, 0].rearrange("(c r) f -> r c f", c=4)
                src = ob[0:128, :].rearrange("r (c f) -> r c f", c=4)
                nc.sync.dma_start(out=dst, in_=src)

            def tail_prior(b0, sm):
                fps = ps.tile([128, BF], F32, tag="ps")
                nc.tensor.matmul(fps[0:8, :], W("prior_a"), sm[0:16, :],
                                 start=True, stop=False)
                nc.tensor.matmul(fps[0:8, :], W("prior_b"), sm[0:8, :],
                                 start=False, stop=True)
                tail(b0, fps)

            for _rep in range(repeat):
                ssall = nrm.tile([65, BF * nt], F32, tag="ssall")
                lss = nrm.tile([65, BF * nt], BF16, tag="lss")
                sall = nrm.tile([65, BF * nt], BF16, tag="sall")

                a1s, a2s, sms, ds = [], [], [], []

                # ---- phase A ----
                for t in range(nt):
                    b0 = t * BF
                    a1 = inA.tile([128, BF], BF16, tag="a1")
                    a2 = inA.tile([128, BF], BF16, tag="a2")
                    sm = inA.tile([128, BF], BF16, tag="sm")
                    nc.sync.dma_start(out=a1[:], in_=A1[b0:b0 + BF, :], transpose=True)
                    nc.sync.dma_start(out=a2[:], in_=A2[b0:b0 + BF, :], transpose=True)
                    nc.sync.dma_start(out=sm[:], in_=SM[b0:b0 + BF, :], transpose=True)
                    a1s.append(a1); a2s.append(a2); sms.append(sm)

                    if stage < 1:
                        ds.append(None)
                        continue
                    m1y = ps.tile([128, BF], F32, tag="ps")
                    nc.tensor.matmul(m1y[0:8, :], W("m1y_a"), sm[0:16, :],
                                     start=True, stop=False)
                    nc.tensor.matmul(m1y[0:8, :], W("m1y_b"), sm[0:8, :],
                                     start=False, stop=True)

                    d = inA.tile([128, BF], BF16, tag="d")
                    ds.append(d)
                    nc.gpsimd.memset(d[:], 0.0)
                    nc.vector.tensor_sub(d[0:8, :], sm[64:72, :], m1y[0:8, :])
                    nc.vector.tensor_sub(d[32:48, :], sm[96:112, :], a2[96:112, :])
                    nc.vector.tensor_sub(d[64:72, :], sm[64:72, :], a2[64:72, :])

                    sq = sbT.tile([128, BF], BF16, tag="sq")
                    nc.vector.tensor_mul(sq[0:128, :], d[0:128, :], d[0:128, :])
                    ss = ps.tile([128, BF], F32, tag="ps")
                    nc.tensor.matmul(ss[0:65, :], W("mm2"), sq[0:128, :])
                    nc.vector.tensor_scalar_add(
                        ssall[0:65, b0:b0 + BF], ss[0:65, :], 1e-30)

                if stage >= 2:
                    nc.scalar.activation(lss[0:65, :], ssall[0:65, :], AF.Ln)
                    nc.scalar.activation(sall[0:65, :], lss[0:65, :], AF.Exp,
                                         scale=-0.5)

                # ---- phase B ----
                for t in range(nt):
                    b0 = t * BF
                    a1, a2, sm, d = a1s[t], a2s[t], sms[t], ds[t]
                    if stage < 3:
                        tail_prior(b0, sm)
                        continue

                    rs = ps.tile([128, BF], F32, tag="ps")
                    nc.tensor.matmul(rs[0:128, :], W("mm3"), sall[0:65, b0:b0 + BF])
                    nd = sbT.tile([128, BF], BF16, tag="nd")
                    nc.vector.tensor_mul(nd[0:128, :], d[0:128, :], rs[0:128, :])
                    if stage < 4:
                        tail_prior(b0, sm)
                        continue

                    f56 = ps.tile([128, BF], F32, tag="ps")
                    nc.tensor.matmul(f56[0:128, :], W("f56"), nd[0:128, :])
                    x56 = sbT.tile([128, BF], BF16, tag="x56")
                    nc.scalar.activation(x56[0:128, :], f56[0:128, :], AF.Relu,
                                         bias=Bv("f56b"))
                    if stage < 5:
                        tail_prior(b0, sm)
                        continue
                    f7 = ps.tile([128, BF], F32, tag="ps")
                    nc.tensor.matmul(f7[0:128, :], W("f7"), nd[0:128, :])
                    x7 = sbT.tile([128, BF], BF16, tag="x7")
                    nc.scalar.activation(x7[0:128, :], f7[0:128, :], AF.Relu,
                                         bias=Bv("f7b"))
                    if stage < 6:
                        tail_prior(b0, sm)
                        continue

                    def gru(g, xrhs, h_mm, h_el, hbase, hprime_out):
                        rz = ps.tile([128, BF], F32, tag="ps")
                        nx = len(xrhs)
                        for i, (suf, rhs) in enumerate(xrhs):
                            nc.tensor.matmul(rz[0:128, :], W(f"{g}_rz_{suf}"), rhs,
                                             start=(i == 0), stop=False)
                        nc.tensor.matmul(rz[0:128, :], W(f"{g}_rz_h"), h_mm,
                                         start=False, stop=True)
                        pnt = ps.tile([128, BF], F32, tag="ps")
                        for i, (suf, rhs) in enumerate(xrhs):
                            nc.tensor.matmul(pnt[0:64, :], W(f"{g}_n_{suf}"), rhs,
                                             start=(i == 0), stop=(i == nx - 1))
                        bpt = ps.tile([128, BF], F32, tag="ps")
                        nc.tensor.matmul(bpt[0:64, :], W(f"{g}_n_h"), h_mm)
                        rzs = sbT.tile([128, BF], BF16, tag="rzs")
                        nc.scalar.activation(rzs[0:128, :], rz[0:128, :], AF.Sigmoid,
                                             bias=Bv(f"rzb_{g}"))
                        tt = sbT.tile([128, BF], BF16, tag="tt")
                        nc.vector.scalar_tensor_tensor(
                            tt[0:64, :], bpt[0:64, :], Bv(f"bhhn_{g}"),
                            rzs[0:64, :], op0=AL.add, op1=AL.mult)
                        ww = sbT.tile([128, BF], BF16, tag="ww")
                        nc.vector.tensor_add(ww[0:64, :], pnt[0:64, :], tt[0:64, :])
                        nt_ = sbT.tile([128, BF], BF16, tag="nt")
                        nb = hbase
                        nc.scalar.activation(nt_[nb:nb + 64, :], ww[0:64, :], AF.Tanh,
                                             bias=Bv(f"nb_{g}"))
                        dt = sbT.tile([128, BF], BF16, tag="dt")
                        nc.vector.tensor_sub(dt[64:128, :], h_el, nt_[nb:nb + 64, :])
                        et = sbT.tile([128, BF], BF16, tag="et")
                        nc.vector.tensor_mul(et[nb:nb + 64, :], rzs[64:128, :],
                                             dt[64:128, :])
                        nc.vector.tensor_add(hprime_out[0:64, :], nt_[nb:nb + 64, :],
                                             et[nb:nb + 64, :])

                    # pieces with rhs above base 0 use full-height K-padded lhsT
                    hq = sbH.tile([128, BF], BF16, tag="hq")
                    gru("Q", [("x", x56[0:40, :])], a1[0:64, :], a1[0:64, :], 0, hq)
                    hsig = sbH.tile([128, BF], BF16, tag="hsig")
                    gru("Sig", [("x1", hq[0:64, :]), ("x2", x56[0:128, :])],
                        a1[0:128, :], a1[64:128, :], 64, hsig)
                    f1 = ps.tile([128, BF], F32, tag="ps")
                    nc.tensor.matmul(f1[0:64, :], W("fc1"), hsig[0:64, :])
                    x1 = sbT.tile([128, BF], BF16, tag="x1")
                    nc.scalar.activation(x1[0:64, :], f1[0:64, :], AF.Relu,
                                         bias=Bv("f1b"))
                    hs = sbH.tile([128, BF], BF16, tag="hs")
                    gru("S", [("x1", x1[0:64, :]), ("x2", x7[0:80, :])],
                        a2[0:64, :], a2[0:64, :], 0, hs)
                    if stage < 7:
                        tail_prior(b0, sm)
                        continue

                    fco = ps.tile([128, BF], F32, tag="ps")
                    for c in range(4):
                        hp = ps.tile([128, BF], F32, tag="ps")
                        nc.tensor.matmul(hp[0:128, :], W(f"fc2a_sig{c}"),
                                         hsig[0:64, :], start=True, stop=False)
                        nc.tensor.matmul(hp[0:128, :], W(f"fc2a_s{c}"),
                                         hs[0:64, :], start=False, stop=True)
                        h2 = sbT.tile([128, BF], BF16, tag="h2")
                        nc.scalar.activation(h2[0:128, :], hp[0:128, :], AF.Relu,
                                             bias=Bv(f"hidb{c}"))
                        nc.tensor.matmul(fco[0:64, :], W(f"fc2b{c}"), h2[0:128, :],
                                         start=(c == 0), stop=(c == 3))

                    dyrp = ps.tile([128, BF], F32, tag="ps")
                    nc.tensor.matmul(dyrp[0:64, :], W("dyrep"), d[0:8, :])
                    dys = sbT.tile([128, BF], BF16, tag="dys")
                    nc.scalar.activation(dys[0:64, :], dyrp[0:64, :], AF.Copy)
                    prd = sbT.tile([128, BF], BF16, tag="prd")
                    nc.vector.scalar_tensor_tensor(
                        prd[0:64, :], fco[0:64, :], Bv("b2"), dys[0:64, :],
                        op0=AL.add, op1=AL.mult)

                    fps = ps.tile([128, BF], F32, tag="ps")
                    nc.tensor.matmul(fps[0:8, :], W("prior_a"), sm[0:16, :],
                                     start=True, stop=False)
                    nc.tensor.matmul(fps[0:8, :], W("prior_b"), sm[0:8, :],
                                     start=False, stop=False)
                    nc.tensor.matmul(fps[0:8, :], W("final"), prd[0:64, :],
                                     start=False, stop=True)
                    tail(b0, fps)

    nc.compile()
    return nc


def _get_built(bc, inputs):
    key = bc
    if key not in _cached:
        wb, wf = _prep_weights(inputs)
        nc = build(bc, wb, wf)
        _cached[key] = (nc, wb, wf)
    return _cached[key]


def run(inputs, trace=False):
    from concourse.bass_utils import run_bass_kernel_spmd

    nc, _, _ = _get_built(BC, inputs)
    # rebuild images from the *current* inputs (block layout is static)
    wb, wf = _prep_weights(inputs)
    wbi = wb.image()
    wfi = wf.image()
    in_maps = []
    for c in range(NCORES):
        a1, a2, sm = _prep_batch(inputs, c * BC, (c + 1) * BC)
        in_maps.append({"A1": a1, "A2": a2, "SM": sm, "WB": wbi, "WF": wfi})
    res = run_bass_kernel_spmd(nc, in_maps, core_ids=list(range(NCORES)),
                               trace=trace)
    outs = [res.results[c]["OUT"] for c in range(NCORES)]
    return np.concatenate(outs, axis=0), res


def kernel(**inputs):
    return run(inputs)[0]

